# revision 2
# baseline (speedup 1.0000x reference)
"""BioBertNerAdvanced fused kernel for 8 TRN2 NeuronCores (pure data parallel over batch).

Pipeline per core (8 batches):
  - word_bert scatter ("first subword wins") as one-hot matmul on TensorE:
      word_bert^T[h, w] = sum_s bert[s, h] * M[s, w],  M[s, w] = (wid[s]==w)&first[s]
  - char embeddings via GPSIMD ap_gather into feature-major layout with
    4 partition-replicas (rows 32r) feeding tile_position-packed conv taps
  - conv as K=30 tap matmuls accumulated in PSUM, relu+bias on ScalarE,
    max-over-time as a bf16 tensor_tensor tree on VectorE
  - pos embeddings via ap_gather
  - MLP1 accumulated from the three feature groups per hidden chunk, MLP2,
    PE transpose of emissions, DMA out
"""

import os

import numpy as np
import ml_dtypes

import bass_rust
import concourse.bass as bass
import concourse.tile as tile
from concourse import mybir
from concourse.bass_utils import run_bass_kernel_spmd
from concourse.tile import ScopedClock


def _split_drain_and_barrier(self, tick_clock, wait_clock):
    """TileContext tail-drain emits one instruction with a sem wait per
    in-flight proc; walrus rejects >2 sync waits on one instruction. Spread
    the waits over SP nops (program order on SP makes the drain safe)."""
    gc = tick_clock.global_clock
    vals = list(gc)
    for i, v in enumerate(vals):
        if v > 0:
            part = [0] * len(vals)
            part[i] = v
            nop = self.nc.sync.nop()
            wait_clock.add_sem_waits(
                nop.ins, ScopedClock({None: bass_rust.VectorClock(part)})
            )
    drain_inst = self.nc.sync.drain()
    wait_clock.add_sem_waits(
        drain_inst.ins, ScopedClock({None: gc}), cur_clock=ScopedClock({None: gc})
    )
    self.nc.all_engine_barrier()
    assert self.sems is not None
    popped = self.nc._tile_sem_poison_stack.pop()
    assert popped is self._sem_poison
    self.nc.clear_and_free_semaphores(list(self.sems.allocated().values()))
    self.nc.all_engine_barrier()


tile.TileContext._drain_and_barrier = _split_drain_and_barrier

WAIT_LIMIT = 1


def _split_excess_waits(nc):
    """Walrus rejects instructions carrying more than WAIT_LIMIT sync waits.
    Spill the excess onto same-engine nops inserted immediately before the
    instruction (engine FIFO order makes the waits equivalent)."""
    for bb in nc.main_func.blocks:
        insts = bb.instructions
        out = []
        for ins in insts:
            si = ins.sync_info
            ow = list(si.on_wait) if si is not None and si.on_wait else []
            if len(ow) > WAIT_LIMIT:
                excess, keep = ow[:-WAIT_LIMIT], ow[-WAIT_LIMIT:]
                for i in range(0, len(excess), WAIT_LIMIT):
                    grp = excess[i:i + WAIT_LIMIT]
                    nop = nc.engines[ins.engine].nop(nofuse=True)
                    # nop() appended itself to the current bb; relocate it
                    for bb2 in nc.main_func.blocks:
                        if bb2.instructions and bb2.instructions[-1] is nop.ins:
                            bb2.instructions.pop()
                            break
                    nop.ins.sync_info = mybir.SyncInfo(on_wait=grp, on_update=[])
                    out.append(nop.ins)
                si.on_wait = keep
            out.append(ins)
        insts[:] = out

# problem dims
B, S, H = 64, 512, 768
W, LC = 256, 16
CV, CE, NF = 256, 30, 50
NPOS, PEM = 20, 25
HID, NL = 256, 9
N_CORES = 8
BB = B // N_CORES          # batches per core
TP = 20                    # padded char pitch per word (2 + 16 + 2)
NP = W * TP                # gathered char stream length per batch
SCH = 4                    # s-chunks of 128
HCH = 6                    # h-chunks of 128

F32 = mybir.dt.float32
F32R = mybir.dt.float32r
BF16 = mybir.dt.bfloat16
I16 = mybir.dt.int16

T_OUT = {2: 17, 3: 16, 4: 17}
T0 = {2: 1, 3: 1, 4: 0}     # 2 - k//2
KSLOT = {2: 0, 3: 1, 4: 2}
# word-chunking of conv N dim so each chunk fits one PSUM bank (<=512 f32)
CHUNKS17 = [(i * 30, 30) for i in range(8)] + [(240, 16)]
CHUNKS16 = [(i * 32, 32) for i in range(8)]
CHUNKS = {2: CHUNKS17, 3: CHUNKS16, 4: CHUNKS17}

RELU = mybir.ActivationFunctionType.Relu
IDENT = mybir.ActivationFunctionType.Identity
MAX = mybir.AluOpType.max
ISEQ = mybir.AluOpType.is_equal
MULT = mybir.AluOpType.mult
NEQ = mybir.AluOpType.not_equal


def build_nc():
    nc = bass.Bass("TRN2", target_bir_lowering=False, debug=False)

    bert_d = nc.dram_tensor("bert", [BB, S, H], BF16, kind="ExternalInput")
    wid_s_d = nc.dram_tensor("wid_s", [128, BB * SCH], F32, kind="ExternalInput")
    wid_p_d = nc.dram_tensor("wid_p", [128, BB * SCH], F32, kind="ExternalInput")
    iota_d = nc.dram_tensor("iota_w", [128, W], BF16, kind="ExternalInput")
    cflat_d = nc.dram_tensor("cflat", [1, BB * NP], BF16, kind="ExternalInput")
    pflat_d = nc.dram_tensor("pflat", [1, BB * W], BF16, kind="ExternalInput")
    ones_d = nc.dram_tensor("ones", [1, 128], BF16, kind="ExternalInput")
    iotav_d = nc.dram_tensor("iotav", [128, 2], F32, kind="ExternalInput")
    iota20_d = nc.dram_tensor("iota20", [NPOS, 1], F32, kind="ExternalInput")
    cemb_d = nc.dram_tensor("cemb", [128, 2, CE], BF16, kind="ExternalInput")
    petab_d = nc.dram_tensor("petab", [NPOS, PEM], BF16, kind="ExternalInput")
    cw_d = nc.dram_tensor("conv_w", [128, 3 * NF], BF16, kind="ExternalInput")
    cb_d = nc.dram_tensor("conv_b", [NF, 3], F32, kind="ExternalInput")
    whb_d = nc.dram_tensor("whb", [128, HCH, HID], BF16, kind="ExternalInput")
    whc_d = nc.dram_tensor("whc", [NF, 3, HID], BF16, kind="ExternalInput")
    whp_d = nc.dram_tensor("whp", [PEM, HID], BF16, kind="ExternalInput")
    bh_d = nc.dram_tensor("bh", [128, 2], F32, kind="ExternalInput")
    wc_d = nc.dram_tensor("wc", [128, 2, NL], BF16, kind="ExternalInput")
    bc_d = nc.dram_tensor("bc", [NL, 1], F32, kind="ExternalInput")
    id_d = nc.dram_tensor("ident", [NL, NL], F32, kind="ExternalInput")
    out_d = nc.dram_tensor("out", [BB, W, NL], F32, kind="ExternalOutput")
    dbg = os.environ.get("KDBG") == "1"
    if dbg:
        dbg_wb = nc.dram_tensor("dbg_wb", [128, HCH, W], F32, kind="ExternalOutput")
        dbg_x = nc.dram_tensor("dbg_x", [128, NP], BF16, kind="ExternalOutput")
        dbg_crep = nc.dram_tensor("dbg_crep", [128, NP], BF16, kind="ExternalOutput")
        dbg_cf = nc.dram_tensor("dbg_cf", [NF, 3, W], BF16, kind="ExternalOutput")
        dbg_pf = nc.dram_tensor("dbg_pf", [PEM, W], BF16, kind="ExternalOutput")
        dbg_hid = nc.dram_tensor("dbg_hid", [128, 2, W], BF16, kind="ExternalOutput")
        dbg_m = nc.dram_tensor("dbg_m", [128, SCH, W], F32, kind="ExternalOutput")

    with tile.TileContext(nc) as tc:
        with (
            tc.tile_pool(name="consts", bufs=1) as cpool,
            tc.tile_pool(name="bert", bufs=2) as bpool,
            tc.tile_pool(name="mmat", bufs=3) as mpool,
            tc.tile_pool(name="x", bufs=2) as xpool,
            tc.tile_pool(name="work", bufs=2) as wpool,
            tc.tile_pool(name="tree", bufs=1) as tpool,
            tc.tile_pool(name="y", bufs=1) as ypool,
            tc.tile_pool(name="oh", bufs=1) as opool,
            tc.tile_pool(name="psum_wb", bufs=1, space="PSUM") as ps_wb,
            tc.tile_pool(name="psum_conv", bufs=2, space="PSUM") as ps_conv,
            tc.tile_pool(name="psum_se", bufs=1, space="PSUM") as ps_se,
            tc.tile_pool(name="psum_scr", bufs=2, space="PSUM") as ps_scr,
        ):
            # ---- constants / params to SBUF ----
            wid_s = cpool.tile([128, BB * SCH], F32)
            nc.gpsimd.dma_start(wid_s[:], wid_s_d[:])
            wid_p = cpool.tile([128, BB * SCH], F32)
            nc.gpsimd.dma_start(wid_p[:], wid_p_d[:])
            iota_w = cpool.tile([128, W], BF16)
            nc.gpsimd.dma_start(iota_w[:], iota_d[:])
            cflat = cpool.tile([1, BB * NP], BF16)
            nc.gpsimd.dma_start(cflat[:], cflat_d[:])
            pflat = cpool.tile([1, BB * W], BF16)
            nc.gpsimd.dma_start(pflat[:], pflat_d[:])
            ones = cpool.tile([1, 128], BF16)
            nc.gpsimd.dma_start(ones[:], ones_d[:])
            iotav = cpool.tile([128, 2], F32)
            nc.gpsimd.dma_start(iotav[:], iotav_d[:])
            iota20 = cpool.tile([NPOS, 1], F32)
            nc.gpsimd.dma_start(iota20[:], iota20_d[:])
            cemb = cpool.tile([128, 2, CE], BF16)
            nc.gpsimd.dma_start(cemb[:], cemb_d[:])
            petab = cpool.tile([NPOS, PEM], BF16)
            nc.gpsimd.dma_start(petab[:], petab_d[:])
            cw = cpool.tile([128, 3 * NF], BF16)
            nc.gpsimd.dma_start(cw[:], cw_d[:])
            cb = cpool.tile([NF, 3], F32)
            nc.gpsimd.dma_start(cb[:], cb_d[:])
            whb = cpool.tile([128, HCH, HID], BF16)
            nc.gpsimd.dma_start(whb[:], whb_d[:])
            whc = cpool.tile([NF, 3, HID], BF16)
            nc.gpsimd.dma_start(whc[:], whc_d[:])
            whp = cpool.tile([PEM, HID], BF16)
            nc.gpsimd.dma_start(whp[:], whp_d[:])
            bh = cpool.tile([128, 2], F32)
            nc.gpsimd.dma_start(bh[:], bh_d[:])
            wc = cpool.tile([128, 2, NL], BF16)
            nc.gpsimd.dma_start(wc[:], wc_d[:])
            bc = cpool.tile([NL, 1], F32)
            nc.gpsimd.dma_start(bc[:], bc_d[:])
            iden = cpool.tile([NL, NL], F32)
            nc.gpsimd.dma_start(iden[:], id_d[:])

            # first-subword mask in s-partition layout, one op for all batches
            first = cpool.tile([128, BB * SCH], F32)
            nc.vector.tensor_tensor(first[:], wid_s[:], wid_p[:], op=NEQ)

            for b in range(BB):
                # ---- load bert s-chunks ----
                bert_k = []
                for k in range(SCH):
                    t = bpool.tile([128, H], BF16, tag="bert")
                    nc.gpsimd.dma_start(t[:], bert_d[b, 128 * k:128 * (k + 1), :])
                    bert_k.append(t)

                # ---- char embeddings: broadcast chars -> one-hot -> embed ----
                crep = opool.tile([128, NP], BF16, tag="crep")
                nc.gpsimd.dma_start(
                    crep[:],
                    cflat_d[0:1, b * NP:(b + 1) * NP].broadcast_to((128, NP)),
                )
                oh = []
                for c in range(2):
                    o = opool.tile([128, NP], BF16, tag=f"oh{c}")
                    nc.vector.tensor_scalar(
                        o[:], crep[:], iotav[:, c:c + 1], None, op0=ISEQ
                    )
                    oh.append(o)
                X = xpool.tile([128, NP], BF16)
                for sl in range(NP // 512):
                    ps = ps_scr.tile([128, 512], F32, tag="scr")
                    for c in range(2):
                        nc.tensor.matmul(
                            ps[0:CE, :], cemb[:, c, :],
                            oh[c][:, 512 * sl:512 * (sl + 1)],
                            start=(c == 0), stop=(c == 1),
                        )
                    nc.vector.tensor_copy(X[0:CE, 512 * sl:512 * (sl + 1)], ps[0:CE, :])
                if dbg and b == 0:
                    nc.gpsimd.dma_start(dbg_crep[:], crep[:])
                    nc.gpsimd.dma_start(dbg_x[:], X[:])
                # shifted tap replicas at 30-row pitch (for K=30k conv matmuls)
                for j in range(1, 4):
                    nc.gpsimd.dma_start(
                        X[CE * j:CE * (j + 1), 0:NP - j], X[0:CE, j:NP]
                    )

                # ---- pos embeddings (same one-hot trick, tiny) ----
                prep_t = wpool.tile([NPOS, W], BF16, tag="prep")
                nc.gpsimd.dma_start(
                    prep_t[:],
                    pflat_d[0:1, b * W:(b + 1) * W].broadcast_to((NPOS, W)),
                )
                ohp = wpool.tile([NPOS, W], BF16, tag="ohp")
                nc.vector.tensor_scalar(
                    ohp[:], prep_t[:], iota20[:, 0:1], None, op0=ISEQ
                )
                ps_p2 = ps_scr.tile([128, 512], F32, tag="scr")
                nc.tensor.matmul(ps_p2[0:PEM, 0:W], petab[:], ohp[:],
                                 start=True, stop=True)
                pf = wpool.tile([PEM, W], BF16, tag="pf")
                nc.scalar.copy(pf[:], ps_p2[0:PEM, 0:W])

                # ---- scatter: word_bert^T via one-hot matmuls ----
                wb_ps = ps_wb.tile([128, HCH * W], F32)
                for k in range(SCH):
                    m_t = mpool.tile([128, W], BF16, tag="m")
                    nc.vector.tensor_scalar(
                        m_t[:], iota_w[:],
                        wid_s[:, b * SCH + k:b * SCH + k + 1],
                        first[:, b * SCH + k:b * SCH + k + 1],
                        op0=ISEQ, op1=MULT,
                    )
                    if dbg and b == 0:
                        pass  # dbg_m tap disabled (bf16)
                    for j in range(HCH):
                        nc.tensor.matmul(
                            wb_ps[:, j * W:(j + 1) * W],
                            bert_k[k][:, j * 128:(j + 1) * 128],
                            m_t[:],
                            start=(k == 0 and j % 2 == 0),
                            stop=(k == SCH - 1 and j % 2 == 1),
                            skip_group_check=True,
                        )

                # ---- evac word_bert + MLP1 bert part ----
                wb_sb = opool.tile([128, HCH, W], BF16, tag="wb")
                for j in range(HCH):
                    nc.vector.tensor_copy(wb_sb[:, j, :], wb_ps[:, j * W:(j + 1) * W])

                if dbg and b == 0:
                    pass  # dbg_wb tap disabled (bf16)
                hid_ps = ps_se.tile([128, 2 * W], F32, tag="se")
                for hc in range(2):
                    for j in range(HCH):
                        nc.tensor.matmul(
                            hid_ps[:, hc * W:(hc + 1) * W],
                            whb[:, j, hc * 128:(hc + 1) * 128],
                            wb_sb[:, j, :],
                            start=(hc == 0 and j == 0), stop=False,
                            skip_group_check=True,
                        )

                # ---- conv + relu/bias evac ----
                X3 = X[:].rearrange("p (w t) -> p w t", t=TP)
                y_sb = {}
                for k in (4, 3, 2):
                    tk = T_OUT[k]
                    y = ypool.tile([NF, W, tk], BF16, tag=f"y{k}")
                    y_sb[k] = y
                    for (w0, wn) in CHUNKS[k]:
                        pc = ps_conv.tile([128, 512], F32, tag="pc")
                        pc3 = pc[:, 0:wn * tk].rearrange("p (w t) -> p w t", t=tk)
                        nc.tensor.matmul(
                            pc3[0:NF, :, :],
                            cw[0:CE * k, KSLOT[k] * NF:(KSLOT[k] + 1) * NF],
                            X3[0:CE * k, w0:w0 + wn, T0[k]:T0[k] + tk],
                            start=True, stop=True,
                        )
                        nc.scalar.activation(
                            y[0:NF, w0:w0 + wn, :], pc3[0:NF, :, :],
                            RELU, bias=cb[:, KSLOT[k]:KSLOT[k] + 1], scale=1.0,
                        )

                # ---- max over time (bf16 tree on DVE) ----
                cf_sb = wpool.tile([NF, 3, W], BF16, tag="cf")
                for k in (4, 3, 2):
                    tk = T_OUT[k]
                    y = y_sb[k]
                    m8 = tpool.tile([NF, W, 8], BF16, tag="m8")
                    nc.vector.tensor_tensor(m8[:], y[:, :, 0:8], y[:, :, 8:16], op=MAX)
                    m4 = tpool.tile([NF, W, 4], BF16, tag="m4")
                    nc.vector.tensor_tensor(m4[:], m8[:, :, 0:4], m8[:, :, 4:8], op=MAX)
                    m2 = tpool.tile([NF, W, 2], BF16, tag="m2")
                    nc.vector.tensor_tensor(m2[:], m4[:, :, 0:2], m4[:, :, 2:4], op=MAX)
                    if tk == 17:
                        m1 = tpool.tile([NF, W, 1], BF16, tag="m1")
                        nc.vector.tensor_tensor(m1[:], m2[:, :, 0:1], m2[:, :, 1:2], op=MAX)
                        nc.vector.tensor_tensor(
                            cf_sb[:, KSLOT[k], :].rearrange("p (w o) -> p w o", o=1),
                            m1[:], y[:, :, 16:17], op=MAX,
                        )
                    else:
                        nc.vector.tensor_tensor(
                            cf_sb[:, KSLOT[k], :].rearrange("p (w o) -> p w o", o=1),
                            m2[:, :, 0:1], m2[:, :, 1:2], op=MAX,
                        )

                if dbg and b == 0:
                    nc.gpsimd.dma_start(dbg_cf[:], cf_sb[:])
                    nc.gpsimd.dma_start(dbg_pf[:], pf[:])
                # ---- MLP1 cf + pf parts (close accumulation) ----
                for hc in range(2):
                    for k in (2, 3, 4):
                        nc.tensor.matmul(
                            hid_ps[:, hc * W:(hc + 1) * W],
                            whc[:, KSLOT[k], hc * 128:(hc + 1) * 128],
                            cf_sb[:, KSLOT[k], :],
                            start=False, stop=False,
                            skip_group_check=True,
                        )
                    nc.tensor.matmul(
                        hid_ps[:, hc * W:(hc + 1) * W],
                        whp[:, hc * 128:(hc + 1) * 128],
                        pf[:, :],
                        start=False, stop=(hc == 1),
                        skip_group_check=True,
                    )

                # ---- hidden relu -> bf16 ----
                hid_sb = wpool.tile([128, 2, W], BF16, tag="hid")
                for hc in range(2):
                    nc.scalar.activation(
                        hid_sb[:, hc, :], hid_ps[:, hc * W:(hc + 1) * W],
                        RELU, bias=bh[:, hc:hc + 1], scale=1.0,
                    )

                if dbg and b == 0:
                    nc.gpsimd.dma_start(dbg_hid[:], hid_sb[:])
                # ---- MLP2 + bias ----
                em_ps = ps_se.tile([128, W + 2 * NL], F32, tag="se")
                for hc in range(2):
                    nc.tensor.matmul(
                        em_ps[0:NL, 0:W], wc[:, hc, :], hid_sb[:, hc, :],
                        start=(hc == 0), stop=(hc == 1),
                        skip_group_check=True,
                    )
                em_sb = wpool.tile([NL, W], F32, tag="em")
                nc.scalar.activation(em_sb[:], em_ps[0:NL, 0:W], IDENT,
                                     bias=bc[:, 0:1], scale=1.0)

                # ---- transpose to word-major + store ----
                for c in range(2):
                    nc.tensor.matmul(
                        em_ps[:, W + NL * c:W + NL * (c + 1)],
                        em_sb[:, 128 * c:128 * (c + 1)],
                        iden[:],
                        is_transpose=True,
                        start=(c == 0), stop=(c == 1),
                        skip_group_check=True,
                    )
                emt = wpool.tile([128, 2, NL], F32, tag="emt")
                nc.scalar.copy(emt[:].rearrange("p c l -> p (c l)"),
                               em_ps[:, W:W + 2 * NL])
                nc.gpsimd.dma_start(
                    out_d[b].rearrange("(c p) l -> p c l", p=128), emt[:]
                )

    _split_excess_waits(nc)
    return nc


def _prep_core(inputs, c):
    """Build the per-core input map (pure slicing / layout / dtype prep)."""
    f32 = np.float32
    bsl = slice(c * BB, (c + 1) * BB)
    bert = np.ascontiguousarray(inputs["bert_hidden"][bsl], dtype=ml_dtypes.bfloat16)
    wid = np.asarray(inputs["word_ids"][bsl], dtype=np.int64)
    cid = np.asarray(inputs["char_ids"][bsl], dtype=np.int64)
    pid = np.asarray(inputs["pos_ids"][bsl], dtype=np.int64)

    # word ids in s-partition layout (+ shifted-by-one copy)
    wid_s = np.empty((128, BB * SCH), f32)
    wid_p = np.empty((128, BB * SCH), f32)
    for b in range(BB):
        for k in range(SCH):
            seg = wid[b, 128 * k:128 * (k + 1)]
            wid_s[:, b * SCH + k] = seg
            prev = np.empty(128, f32)
            if k == 0:
                prev[0] = -1.0
                prev[1:] = seg[:-1]
            else:
                prev[0] = wid[b, 128 * k - 1]
                prev[1:] = seg[:-1]
            wid_p[:, b * SCH + k] = prev

    iota_w = np.broadcast_to(np.arange(W), (128, W)).astype(ml_dtypes.bfloat16)

    bf16 = ml_dtypes.bfloat16
    # char values in a padded pitch-20 per-word stream (pad slots get CV=256,
    # which one-hots to all-zero -> zero embedding = conv zero padding)
    cflat = np.empty((1, BB * NP), bf16)
    for b in range(BB):
        tmpl = np.full((W, TP), CV, np.int64)
        tmpl[:, 2:2 + LC] = cid[b]
        cflat[0, b * NP:(b + 1) * NP] = tmpl.reshape(NP).astype(bf16)
    pflat = pid.reshape(1, BB * W).astype(bf16)

    ones = np.ones((1, 128), bf16)
    iotav = np.empty((128, 2), f32)
    iotav[:, 0] = np.arange(128)
    iotav[:, 1] = np.arange(128, 256)
    iota20 = np.arange(NPOS, dtype=f32).reshape(NPOS, 1)
    cemb = np.ascontiguousarray(
        np.asarray(inputs["char_emb"], f32).reshape(2, 128, CE).transpose(1, 0, 2),
        dtype=bf16,
    )  # (128, 2, CE)
    petab = np.asarray(inputs["pos_emb"], f32).astype(bf16)  # (NPOS, PEM)

    # conv weights at 30-row pitch: [30j + c, kslot*50 + f] = w_k[f, c, j]
    conv_w = np.zeros((128, 3 * NF), f32)
    for k in (2, 3, 4):
        wk = np.asarray(inputs[f"conv_w{k}"], f32)  # (NF, CE, k)
        for j in range(k):
            conv_w[CE * j:CE * (j + 1), KSLOT[k] * NF:(KSLOT[k] + 1) * NF] = wk[:, :, j].T
    conv_w = conv_w.astype(bf16)
    conv_b = np.stack(
        [np.asarray(inputs[f"conv_b{k}"], f32) for k in (2, 3, 4)], axis=1
    )  # (NF, 3)

    wh = np.asarray(inputs["W_h"], f32)  # (943, 256)
    whb = wh[:H].reshape(HCH, 128, HID).transpose(1, 0, 2).astype(ml_dtypes.bfloat16)
    whc = np.ascontiguousarray(
        wh[H:H + 3 * NF].reshape(3, NF, HID).transpose(1, 0, 2),
        dtype=ml_dtypes.bfloat16,
    )  # (NF, 3, HID)
    whp = np.ascontiguousarray(wh[H + 3 * NF:], dtype=ml_dtypes.bfloat16)  # (25, 256)
    bh = np.asarray(inputs["b_h"], f32).reshape(2, 128).T.copy()  # (128, 2)
    wcf = np.asarray(inputs["W_c"], f32)  # (256, 9)
    wc = np.ascontiguousarray(
        wcf.reshape(2, 128, NL).transpose(1, 0, 2), dtype=ml_dtypes.bfloat16
    )  # (128, 2, 9)
    bc = np.asarray(inputs["b_c"], f32).reshape(NL, 1)
    ident = np.eye(NL, dtype=f32)

    return dict(
        bert=bert, wid_s=wid_s, wid_p=wid_p, iota_w=iota_w,
        cflat=cflat, pflat=pflat, ones=ones, iotav=iotav, iota20=iota20,
        cemb=cemb, petab=petab,
        conv_w=conv_w, conv_b=conv_b,
        whb=np.ascontiguousarray(whb), whc=whc, whp=whp,
        bh=bh, wc=wc, bc=bc, ident=ident,
    )


_NC_CACHE = {}


def kernel(**inputs) -> np.ndarray:
    if "nc" not in _NC_CACHE:
        _NC_CACHE["nc"] = build_nc()
    nc = _NC_CACHE["nc"]
    in_maps = [_prep_core(inputs, c) for c in range(N_CORES)]
    res = run_bass_kernel_spmd(nc, in_maps, list(range(N_CORES)))
    _NC_CACHE["last_result"] = res
    out = np.concatenate([res.results[c]["out"] for c in range(N_CORES)], axis=0)
    return out.astype(np.float32)

